# revision 1
# baseline (speedup 1.0000x reference)
import sys

sys.path.insert(0, "/opt/trn_rl_repo")

import math

import numpy as np

import concourse.bass as bass
import concourse.mybir as mybir
import concourse.tile as tile
from concourse import bacc
from concourse.bass_utils import run_bass_kernel_spmd
from concourse.masks import make_identity

F32 = mybir.dt.float32
F32R = mybir.dt.float32r
IDENT = mybir.ActivationFunctionType.Identity
EXPF = mybir.ActivationFunctionType.Exp

B, S, D = 8, 1024, 1024
N_H = 16
REL_K = 16
d_k = D // N_H  # 64
N_CORES = 8
MASKVAL = -1e30
NBUF = 6

_CACHE = {}
TRACE = False


def build_module():
    nc = bacc.Bacc("TRN2", detect_race_conditions=False, num_swdge_queues=4)

    xT = nc.dram_tensor("xT", [D, S], F32R, kind="ExternalInput")
    Wqk = nc.dram_tensor("Wqk", [D, 2 * D], F32R, kind="ExternalInput")
    Wv = nc.dram_tensor("Wv", [D, D], F32R, kind="ExternalInput")
    Wp = nc.dram_tensor("Wp", [D, D], F32R, kind="ExternalInput")
    bqk = nc.dram_tensor("bqk", [128, 16], F32, kind="ExternalInput")
    bvp = nc.dram_tensor("bvp", [1, D], F32R, kind="ExternalInput")
    bp = nc.dram_tensor("bp", [1, D], F32R, kind="ExternalInput")
    dlut = nc.dram_tensor("dlut", [d_k, 16], F32R, kind="ExternalInput")
    dlv = nc.dram_tensor("dlv", [16, d_k], F32, kind="ExternalInput")
    selm = nc.dram_tensor("selm", [16, 1024], F32R, kind="ExternalInput")
    zb2init = nc.dram_tensor("zb2init", [128, 160], F32, kind="ExternalInput")
    OUT = nc.dram_tensor("OUT", [S, D], F32, kind="ExternalOutput")

    zb2 = [nc.dram_tensor(f"zb2_{k}", [128, 160], F32) for k in range(8)]
    zdp = [nc.dram_tensor(f"zdp_{k}", [16, 1040], F32) for k in range(2)]
    zb1 = [nc.dram_tensor(f"zb1_{k}", [16, 160], F32) for k in range(NBUF)]
    ewd = [nc.dram_tensor(f"ewd_{k}", [128, 144], F32) for k in range(NBUF)]

    with tile.TileContext(nc) as tc:
        with (
            tc.tile_pool(name="persist", bufs=1) as pers,
            tc.tile_pool(name="small", bufs=5) as small,
            tc.tile_pool(name="ps_big", bufs=1, space="PSUM") as ps_big,
        ):
            # ---- constants ----
            ident = pers.tile([128, 128], F32)
            make_identity(nc, ident[:])
            identR = pers.tile([128, 128], F32R)
            nc.vector.tensor_copy(identR[:], ident[:])
            dlut_sb = pers.tile([128, 16], F32R)
            nc.sync.dma_start(out=dlut_sb[0:64, :], in_=dlut[:])
            nc.sync.dma_start(out=dlut_sb[64:128, :], in_=dlut[:])
            dlv_sb = pers.tile([16, d_k], F32R)
            nc.gpsimd.dma_start(out=dlv_sb[:], in_=dlv[:])
            selm_sb = pers.tile([16, 1024], F32R)
            nc.sync.dma_start(out=selm_sb[:], in_=selm[:])
            bqk_sb = pers.tile([128, 16], F32)
            nc.gpsimd.dma_start(out=bqk_sb[:], in_=bqk[:])
            bvp_sb = pers.tile([1, D], F32R)
            nc.sync.dma_start(out=bvp_sb[:], in_=bvp[:])
            bp_sb = pers.tile([1, D], F32R)
            nc.sync.dma_start(out=bp_sb[:], in_=bp[:])
            ones1f = pers.tile([1, 128], F32)
            nc.vector.memset(ones1f[:], 1.0)
            ones1 = pers.tile([1, 128], F32R)
            nc.vector.tensor_copy(ones1[:], ones1f[:])
            onescol_f = pers.tile([128, 16], F32)
            nc.vector.memset(onescol_f[:], 1.0)

            z16 = pers.tile([16, 160], F32)
            nc.vector.memset(z16[:], 0.0)
            zi_sb = pers.tile([128, 160], F32)
            nc.gpsimd.dma_start(out=zi_sb[:], in_=zb2init[:])
            for k in range(NBUF):
                nc.sync.dma_start(out=zb1[k][:], in_=z16[:])
            for k in range(8):
                nc.sync.dma_start(out=zb2[k][:], in_=zi_sb[:])

            # ---- load xT (f32r) ----
            xT_sb = []
            for d in range(8):
                t = pers.tile([128, S], F32R, tag=f"xT{d}")
                nc.sync.dma_start(out=t[:], in_=xT[128 * d:128 * (d + 1), :])
                xT_sb.append(t)

            # ---- v projection -> vhat_sb (65-stride layout + ones cols) ----
            vhat_sb = [pers.tile([128, 16 * 65], F32R, name=f"vh{jt}", tag=f"vh{jt}")
                       for jt in range(8)]
            with tc.tile_pool(name="wv", bufs=1) as wvp:
                Wv_sb = []
                for d in range(8):
                    t = wvp.tile([128, D], F32R, tag=f"wv{d}")
                    nc.sync.dma_start(out=t[:], in_=Wv[128 * d:128 * (d + 1), :])
                    Wv_sb.append(t)
                for tt in range(8):
                    vt = vhat_sb[tt]
                    ones_ap = bass.AP(tensor=vt[:].tensor, offset=64,
                                      ap=[[16 * 65, 128], [65, 16]])
                    nc.vector.tensor_copy(ones_ap, onescol_f[:])
                    for fc in range(2):
                        ps = ps_big.tile([128, 512], F32, tag="pbig")
                        for d in range(8):
                            nc.tensor.matmul(
                                ps[:],
                                xT_sb[d][:, 128 * tt:128 * (tt + 1)],
                                Wv_sb[d][:, 512 * fc:512 * (fc + 1)],
                                start=(d == 0), stop=False,
                            )
                        nc.tensor.matmul(
                            ps[:],
                            ones1[:],
                            bvp_sb[:, 512 * fc:512 * (fc + 1)],
                            start=False, stop=True,
                        )
                        srcA = bass.AP(tensor=ps[:].tensor,
                                       offset=ps[:].offset,
                                       ap=[[512, 128], [64, 8], [1, 64]])
                        dst = bass.AP(tensor=vt[:].tensor, offset=65 * 8 * fc,
                                      ap=[[16 * 65, 128], [65, 8], [1, 64]])
                        nc.scalar.copy(dst, srcA)

            pair_sb = [pers.tile([128, S], F32R, name=f"pair{hp}", tag=f"pair{hp}")
                       for hp in range(8)]
            denoms = pers.tile([16, S], F32)

            # ---- attention ----
            with (
                tc.tile_pool(name="wqk", bufs=12) as wqkp,
                tc.tile_pool(name="qk", bufs=2) as qkp,
                tc.tile_pool(name="bandp", bufs=2) as bandp,
                tc.tile_pool(name="outtp", bufs=2) as outtp,
                tc.tile_pool(name="eskp", bufs=2) as eskp,
                tc.tile_pool(name="dtp", bufs=1) as dtp,
                tc.tile_pool(name="att", bufs=4) as attp,
                tc.tile_pool(name="dpp", bufs=2) as dpp,
                tc.tile_pool(name="ps_s", bufs=4, space="PSUM") as ps_s,
                tc.tile_pool(name="ps_out", bufs=1, space="PSUM") as ps_out,
                tc.tile_pool(name="ps_sm", bufs=1, space="PSUM") as ps_sm,
            ):
                rot = 0

                def emit_qkproj(hp2):
                    qk_pair = []
                    for sec, ft in ((0, hp2), (1, 8 + hp2)):
                        ws = []
                        for d in range(8):
                            w = wqkp.tile([128, 128], F32R, tag="wqk")
                            nc.gpsimd.dma_start(
                                out=w[:],
                                in_=Wqk[128 * d:128 * (d + 1), 128 * ft:128 * (ft + 1)])
                            ws.append(w)
                        dstt = qkp.tile([128, S], F32R, tag=f"qk{sec}")
                        for tch in range(2):
                            ps = ps_big.tile([128, 512], F32, tag="pbig")
                            for d in range(8):
                                nc.tensor.matmul(
                                    ps[:],
                                    ws[d][:],
                                    xT_sb[d][:, 512 * tch:512 * (tch + 1)],
                                    start=(d == 0), stop=(d == 7),
                                )
                            nc.scalar.activation(dstt[:, 512 * tch:512 * (tch + 1)],
                                                 ps[:], IDENT,
                                                 bias=bqk_sb[:, ft:ft + 1], scale=1.0)
                        qk_pair.append(dstt)
                    return qk_pair

                next_qk = emit_qkproj(0)
                for hp in range(8):
                    qT_pair, kT_pair = next_qk

                    for hh in range(2):
                        h = 2 * hp + hh
                        po = hh * 64
                        qT = qT_pair[po:po + 64, :]
                        kT = kT_pair[po:po + 64, :]

                        # dp matmuls -> dpT [16, 1040]
                        dpT = dpp.tile([16, 1040], F32, tag="dpT")
                        nc.vector.memset(dpT[:, 1024:1040], 0.0)
                        for c in range(2):
                            psdp = ps_s.tile([128, 512], F32, tag="pss")
                            nc.tensor.matmul(psdp[0:16, :],
                                             dlut_sb[po:po + 64, :],
                                             qT[:, 512 * c:512 * (c + 1)],
                                             start=True, stop=True)
                            nc.vector.tensor_copy(dpT[:, 512 * c:512 * (c + 1)],
                                                  psdp[0:16, :])
                        dpSh = dpp.tile([16, 1040], F32, tag="dpSh")
                        zdp_i = h % 2
                        nc.scalar.dma_start(out=zdp[zdp_i][:], in_=dpT[:])
                        srcSh = bass.AP(tensor=zdp[zdp_i][:].tensor, offset=0,
                                        ap=[[1041, 16], [1, 1024]])
                        nc.scalar.dma_start(out=dpSh[:, 0:1024], in_=srcSh)

                        band_tiles = []
                        for jt in range(8):
                            j0 = 128 * jt
                            psd = ps_sm.tile([128, 128], F32, tag="pstr")
                            nc.tensor.transpose(psd[0:128, 0:16],
                                                dpSh[:, j0:j0 + 128],
                                                ident[0:16, 0:16])
                            dpS = small.tile([128, 16], F32, tag="dpS")
                            nc.vector.tensor_copy(dpS[:], psd[0:128, 0:16])
                            dstW = bass.AP(tensor=zb2[jt][:].tensor, offset=0,
                                           ap=[[161, 128], [1, 16]])
                            nc.scalar.dma_start(out=dstW, in_=dpS[:])
                            band = bandp.tile([128, 160], F32, name=f"band{jt}",
                                              tag=f"band{jt}")
                            nc.gpsimd.dma_start(out=band[:], in_=zb2[jt][:])
                            band_tiles.append(band)

                        pso = ps_out.tile([65, 1024], F32, tag="pso")
                        esk_tiles = []

                        for jt in range(8):
                            j0 = 128 * jt
                            wdiag = min(512, S - j0)
                            win = min(144, S - j0)
                            pss0 = ps_s.tile([128, 512], F32, tag="pss")
                            nc.tensor.matmul(pss0[:, 0:wdiag],
                                             kT[:, j0:j0 + 128],
                                             qT[:, j0:j0 + wdiag],
                                             start=True, stop=True)
                            band = band_tiles[jt]
                            expT = attp.tile([128, 1024], F32R, tag="expT")
                            sS = small.tile([128, 144], F32, tag="sS")
                            nc.vector.tensor_add(sS[:, 0:win], pss0[:, 0:win],
                                                 band[:, 0:win])
                            nc.scalar.activation(expT[:, 0:win], sS[:, 0:win], EXPF)
                            zb_i = rot % NBUF
                            rot += 1
                            ew = small.tile([128, 144], F32, tag="ew")
                            if win < 144:
                                nc.vector.memset(ew[:, win:144], 0.0)
                            nc.vector.tensor_copy(ew[:, 0:win], expT[:, 0:win])
                            nc.sync.dma_start(out=ewd[zb_i][:], in_=ew[:])
                            esk = eskp.tile([128, 16], F32, name=f"esk{jt}",
                                            tag=f"esk{jt}")
                            srcR = bass.AP(tensor=ewd[zb_i][:].tensor, offset=0,
                                           ap=[[145, 128], [1, 16]])
                            nc.sync.dma_start(out=esk[:], in_=srcR)
                            esk_tiles.append((esk, zb_i))
                            if wdiag > win:
                                nc.scalar.activation(expT[:, win:wdiag],
                                                     pss0[:, win:wdiag], EXPF)
                            if S - j0 > 512:
                                w1 = S - j0 - 512
                                pss1 = ps_s.tile([128, 512], F32, tag="pss")
                                nc.tensor.matmul(pss1[:, 0:w1],
                                                 kT[:, j0:j0 + 128],
                                                 qT[:, j0 + 512:S],
                                                 start=True, stop=True)
                                nc.scalar.activation(expT[:, 512:512 + w1],
                                                     pss1[:, 0:w1], EXPF)
                            lhs = vhat_sb[jt][:, 65 * h:65 * h + 65]
                            segs = ([(j0, 512), (512, 1024)] if j0 < 512
                                    else [(j0, 1024)])
                            for (a, b2) in segs:
                                nc.tensor.matmul(pso[:, a:b2], lhs,
                                                 expT[:, a - j0:b2 - j0],
                                                 start=(jt == 0), stop=False,
                                                 skip_group_check=True)

                        if hh == 1 and hp < 7:
                            next_qk = emit_qkproj(hp + 1)
                        # phase 2: transposes + shear-out + dt reads
                        dt_tiles = []
                        for jt in range(8):
                            esk, zb_i = esk_tiles[jt]
                            pst = ps_sm.tile([128, 128], F32, tag="pstr")
                            nc.tensor.transpose(pst[0:16, 0:128], esk[:], ident[:])
                            t1 = small.tile([16, 128], F32, tag="t1")
                            nc.vector.tensor_copy(t1[:], pst[0:16, 0:128])
                            dstZ = bass.AP(tensor=zb1[zb_i][:].tensor, offset=0,
                                           ap=[[161, 16], [1, 128]])
                            nc.scalar.dma_start(out=dstZ, in_=t1[:])
                            dt_sb = dtp.tile([16, 160], F32R, name=f"dt{jt}",
                                             tag=f"dt{jt}")
                            nc.gpsimd.dma_start(out=dt_sb[:], in_=zb1[zb_i][:])
                            dt_tiles.append(dt_sb)
                        esk_tiles.clear()

                        # phase 3: DT matmuls
                        for jt in range(8):
                            j0 = 128 * jt
                            win = min(144, S - j0)
                            dt_sb = dt_tiles[jt]
                            a0, b0 = j0, j0 + win
                            dsegs = ([(a0, 512), (512, b0)] if (a0 < 512 < b0)
                                     else [(a0, b0)])
                            for (a, b2) in dsegs:
                                nc.tensor.matmul(pso[0:64, a:b2], dlv_sb[:],
                                                 dt_sb[:, a - j0:b2 - j0],
                                                 start=False,
                                                 stop=(jt == 7 and (a, b2) == dsegs[-1]),
                                                 skip_group_check=True)

                        # evict head result via SBUF (DMA moves partitions)
                        outT_sb = outtp.tile([65, 1024], F32R, tag="outT")
                        nc.scalar.copy(outT_sb[:], pso[:])
                        nc.sync.dma_start(out=pair_sb[hp][po:po + 64, :],
                                          in_=outT_sb[0:64, :])
                        nc.sync.dma_start(out=denoms[h:h + 1, :].bitcast(F32R),
                                          in_=outT_sb[64:65, :])


            # ---- normalize ----
            recip = pers.tile([16, S], F32R)
            with nc.allow_low_precision(reason="f32r rounding for matmul broadcast"):
                nc.vector.reciprocal(recip[:], denoms[:])
            with tc.tile_pool(name="ps_n", bufs=2, space="PSUM") as ps_n:
                for hp in range(8):
                    psb = ps_n.tile([128, 1024], F32, tag="psb")
                    for c in range(2):
                        nc.tensor.matmul(psb[:, 512 * c:512 * (c + 1)],
                                         selm_sb[:, 128 * hp:128 * (hp + 1)],
                                         recip[:, 512 * c:512 * (c + 1)],
                                         start=True, stop=True)
                    nc.vector.tensor_mul(pair_sb[hp][:], pair_sb[hp][:], psb[:])

            # ---- final projection ----
            with (
                tc.tile_pool(name="wp", bufs=1) as wpp,
                tc.tile_pool(name="ps_p", bufs=2, space="PSUM") as ps_p,
                tc.tile_pool(name="outp", bufs=2) as outp,
            ):
                Wp_sb = []
                for d in range(8):
                    t = wpp.tile([128, D], F32R, tag=f"wp{d}")
                    nc.sync.dma_start(out=t[:], in_=Wp[128 * d:128 * (d + 1), :])
                    Wp_sb.append(t)
                for tt in range(8):
                    ps = ps_p.tile([128, 1024], F32, tag="psp")
                    for fc in range(2):
                        for d in range(8):
                            nc.tensor.matmul(
                                ps[:, 512 * fc:512 * (fc + 1)],
                                pair_sb[d][:, 128 * tt:128 * (tt + 1)],
                                Wp_sb[d][:, 512 * fc:512 * (fc + 1)],
                                start=(d == 0), stop=False,
                            )
                            pass
                        nc.tensor.matmul(
                            ps[:, 512 * fc:512 * (fc + 1)],
                            ones1[:],
                            bp_sb[:, 512 * fc:512 * (fc + 1)],
                            start=False, stop=True,
                        )
                    ot = outp.tile([128, 1024], F32, tag="ot")
                    nc.vector.tensor_copy(ot[:], ps[:])
                    nc.sync.dma_start(out=OUT[128 * tt:128 * (tt + 1), :], in_=ot[:])

    nc.compile()
    return nc


def _host_prep(W_attn, b_attn, W_proj, b_proj, lut_k, lut_v):
    scale = 1.0 / math.sqrt(d_k)
    Wqk = np.concatenate([W_attn[:, :D], W_attn[:, D:2 * D] * scale], axis=1)
    bq = b_attn[:D]
    bk = b_attn[D:2 * D] * scale
    bqk_h = np.stack([np.concatenate([bq, bk])[128 * ft:128 * (ft + 1)]
                      for ft in range(16)], axis=1).astype(np.float32)
    bvp_h = (b_attn[2 * D:3 * D] + np.tile(lut_v[0], N_H)).reshape(1, D)
    dlut_h = np.stack([(lut_k[16 - u] - lut_k[0]) * scale for u in range(16)],
                      axis=1).astype(np.float32)
    dlv_h = np.stack([lut_v[16 - u] - lut_v[0] for u in range(16)],
                     axis=0).astype(np.float32)
    selm_h = np.zeros((16, 1024), np.float32)
    for hp in range(8):
        for p in range(128):
            selm_h[2 * hp + p // 64, 128 * hp + p] = 1.0
    zb2_h = np.where(np.arange(160)[None, :] < np.arange(128)[:, None],
                     np.float32(MASKVAL), np.float32(0.0)).astype(np.float32)
    return {
        "Wqk": np.ascontiguousarray(Wqk, np.float32),
        "Wv": np.ascontiguousarray(W_attn[:, 2 * D:3 * D], np.float32),
        "Wp": np.ascontiguousarray(W_proj, np.float32),
        "bqk": bqk_h,
        "bvp": np.ascontiguousarray(bvp_h, np.float32),
        "bp": np.ascontiguousarray(np.asarray(b_proj).reshape(1, D), np.float32),
        "dlut": dlut_h,
        "dlv": dlv_h,
        "selm": selm_h,
        "zb2init": zb2_h,
    }


def kernel(x, W_attn, b_attn, W_proj, b_proj, lut_k, lut_v):
    x = np.asarray(x, np.float32)
    shared = _host_prep(np.asarray(W_attn, np.float32),
                        np.asarray(b_attn, np.float32),
                        np.asarray(W_proj, np.float32),
                        np.asarray(b_proj, np.float32),
                        np.asarray(lut_k, np.float32),
                        np.asarray(lut_v, np.float32))
    if "nc" not in _CACHE:
        _CACHE["nc"] = build_module()
    nc = _CACHE["nc"]
    in_maps = []
    for b in range(N_CORES):
        m = dict(shared)
        m["xT"] = np.ascontiguousarray(x[b].T)
        in_maps.append(m)
    res = run_bass_kernel_spmd(nc, in_maps, list(range(N_CORES)), trace=TRACE)
    _CACHE["last_result"] = res
    out = np.stack([res.results[b]["OUT"] for b in range(N_CORES)], axis=0)
    return out.astype(np.float32)

